# revision 2
# baseline (speedup 1.0000x reference)
"""MoE (top-2 of 8 experts) Trainium2 kernel — expert-parallel across 8 NeuronCores.

Full-input contract: kernel(**inputs) takes the unsharded numpy inputs and
returns the full [B, S, D] output.

Strategy:
  * Host: router (logits -> top-2 -> softmax gates), all-to-all dispatch by
    expert id (gather the tokens routed to each expert, pad to a static
    count), and the final combine (scatter-add of the two gated expert
    outputs per token).
  * Device (one expert per core): y = g * (relu(x @ W1 + b1) @ W2 + b2)
    for that expert's dispatched tokens.  Matmuls run in float32r
    (TF32-like, full PE rate); accumulation is fp32 in PSUM.
    W2 stays SBUF-resident for the whole kernel; W1 is streamed once per
    384-token block. b2 is applied with a rank-1 (K=1) matmul into the
    accumulating PSUM tile, and the gate scale rides the PSUM->SBUF copy
    on the scalar engine.
"""

import numpy as np

import concourse.tile as tile
import concourse.mybir as mybir
from concourse import bacc, bass_utils

B, S, D, F, E, TOPK = 4, 2048, 1024, 4096, 8, 2
T = B * S
P = 128
T_BLK = 384  # tokens per weight pass; 3 token tiles * 2 D-halves = 6 PSUM banks
F32 = mybir.dt.float32
F32R = mybir.dt.float32r
AF = mybir.ActivationFunctionType

_CACHE: dict[int, object] = {}


def _build(n_pad: int):
    """Build + compile the per-core Bass program for n_pad dispatched tokens."""
    nb = n_pad // T_BLK
    nc = bacc.Bacc("TRN2", target_bir_lowering=False, debug=False)

    xT = nc.dram_tensor("xT", (D, n_pad), F32R, kind="ExternalInput")
    w1 = nc.dram_tensor("w1", (D, F), F32R, kind="ExternalInput")
    b1c = nc.dram_tensor("b1c", (P, F // P), F32, kind="ExternalInput")
    w2 = nc.dram_tensor("w2", (F, D), F32R, kind="ExternalInput")
    b2r = nc.dram_tensor("b2r", (D,), F32R, kind="ExternalInput")
    gt = nc.dram_tensor("gt", (P, n_pad // P), F32, kind="ExternalInput")
    y = nc.dram_tensor("y", (n_pad, D), F32, kind="ExternalOutput")

    xT_t = xT.rearrange("(o p) n -> p o n", p=P)       # [128, 8, n_pad]
    w1_t = w1.rearrange("(o p) m -> p o m", p=P)       # [128, 8, 4096]
    w2_t = w2.rearrange("(o p) d -> p o d", p=P)       # [128, 32, 1024]

    FT = F // P  # 32 f tiles
    DT = D // P  # 8 d tiles
    NT = T_BLK // P  # 3 token tiles per block
    DH = D // 512  # 2 output halves

    with tile.TileContext(nc) as tc:
        with (
            tc.tile_pool(name="w2p", bufs=1) as w2p,
            tc.tile_pool(name="const", bufs=1) as constp,
            tc.tile_pool(name="xp", bufs=2) as xp,
            tc.tile_pool(name="w1p", bufs=4) as w1p,
            tc.tile_pool(name="hp", bufs=4) as hp,
            tc.tile_pool(name="op", bufs=6) as op,
            tc.tile_pool(name="ph", bufs=2, space="PSUM") as php,
            tc.tile_pool(name="py", bufs=6, space="PSUM") as pyp,
        ):
            # ---- resident / constant loads ----
            w2_sb = w2p.tile([P, FT, D], F32R)
            nc.sync.dma_start(w2_sb[:], w2_t[:])
            b1_sb = constp.tile([P, FT], F32)
            nc.sync.dma_start(b1_sb[:], b1c[:])
            g_sb = constp.tile([P, n_pad // P], F32)
            nc.sync.dma_start(g_sb[:], gt[:])
            b2_sb = constp.tile([1, D], F32R)
            nc.sync.dma_start(b2_sb[:], b2r[None, :])
            ones_f = constp.tile([1, P], F32)
            ones_sb = constp.tile([1, P], F32R)
            nc.vector.memset(ones_f[:], 1.0)
            nc.vector.tensor_copy(ones_sb[:], ones_f[:])

            for blk in range(nb):
                x_sb = xp.tile([P, DT, T_BLK], F32R)
                nc.sync.dma_start(
                    x_sb[:], xT_t[:, :, blk * T_BLK : (blk + 1) * T_BLK]
                )

                psum_y = [pyp.tile([P, 512], F32, name="py") for j in range(NT * DH)]

                for f in range(FT):
                    w1_sb = w1p.tile([P, DT, P], F32R)
                    nc.sync.dma_start(w1_sb[:], w1_t[:, :, f * P : (f + 1) * P])
                    ph = php.tile([P, T_BLK], F32)
                    for d in range(DT):
                        nc.tensor.matmul(
                            ph[:],
                            w1_sb[:, d],
                            x_sb[:, d],
                            start=(d == 0),
                            stop=(d == DT - 1),
                        )
                    ht = hp.tile([P, T_BLK], F32R)
                    nc.scalar.activation(
                        ht[:], ph[:], AF.Relu, bias=b1_sb[:, f : f + 1], scale=1.0
                    )
                    for t in range(NT):
                        for dh in range(DH):
                            nc.tensor.matmul(
                                psum_y[t * DH + dh][:],
                                ht[:, t * P : (t + 1) * P],
                                w2_sb[:, f, dh * 512 : (dh + 1) * 512],
                                start=(f == 0),
                                stop=False,
                            )

                for t in range(NT):
                    col = blk * NT + t
                    for dh in range(DH):
                        pj = psum_y[t * DH + dh]
                        nc.tensor.matmul(
                            pj[:],
                            ones_sb[:],
                            b2_sb[:, dh * 512 : (dh + 1) * 512],
                            start=False,
                            stop=True,
                        )
                        ot = op.tile([P, 512], F32)
                        nc.scalar.activation(
                            ot[:], pj[:], AF.Copy, scale=g_sb[:, col : col + 1]
                        )
                        nc.sync.dma_start(
                            y[
                                blk * T_BLK + t * P : blk * T_BLK + (t + 1) * P,
                                dh * 512 : (dh + 1) * 512,
                            ],
                            ot[:],
                        )
    nc.compile()
    return nc


def _route(x_flat, Wg, bg):
    """Top-2 routing. Returns (order, counts, offsets, gates_flat, n_pad)."""
    logits = x_flat @ Wg + bg  # [T, E]
    i1 = np.argmax(logits, axis=1)
    v1 = logits[np.arange(T), i1]
    masked = logits.copy()
    masked[np.arange(T), i1] = -np.inf
    i2 = np.argmax(masked, axis=1)
    v2 = masked[np.arange(T), i2]
    # softmax over the two selected logits
    e2 = np.exp(v2 - v1)
    g1 = 1.0 / (1.0 + e2)
    g2 = e2 / (1.0 + e2)
    eid = np.stack([i1, i2], 1).reshape(-1)  # [2T]
    gates = np.stack([g1, g2], 1).reshape(-1).astype(np.float32)
    order = np.argsort(eid, kind="stable")
    counts = np.bincount(eid, minlength=E)
    offsets = np.concatenate([[0], np.cumsum(counts)])
    n_pad = max(T_BLK, int(-(-counts.max() // T_BLK)) * T_BLK)
    return order, counts, offsets, gates, n_pad


def kernel(x, Wg, bg, W1, b1, W2, b2, _trace=False):
    x = np.ascontiguousarray(np.asarray(x, dtype=np.float32))
    Wg = np.asarray(Wg, dtype=np.float32)
    bg = np.asarray(bg, dtype=np.float32)
    W1 = np.asarray(W1, dtype=np.float32)
    b1 = np.asarray(b1, dtype=np.float32)
    W2 = np.asarray(W2, dtype=np.float32)
    b2 = np.asarray(b2, dtype=np.float32)

    x_flat = x.reshape(T, D)
    order, counts, offsets, gates, n_pad = _route(x_flat, Wg, bg)

    if n_pad not in _CACHE:
        _CACHE[n_pad] = _build(n_pad)
    nc = _CACHE[n_pad]

    in_maps = []
    for e in range(E):
        ce = int(counts[e])
        sel = order[offsets[e] : offsets[e] + ce]
        toks = sel >> 1
        xT_e = np.zeros((D, n_pad), dtype=np.float32)
        xT_e[:, :ce] = x_flat[toks].T
        g_e = np.zeros(n_pad, dtype=np.float32)
        g_e[:ce] = gates[sel]
        in_maps.append(
            {
                "xT": xT_e,
                "w1": np.ascontiguousarray(W1[e]),
                "b1c": np.ascontiguousarray(b1[e].reshape(F // P, P).T),
                "w2": np.ascontiguousarray(W2[e]),
                "b2r": np.ascontiguousarray(b2[e]),
                "gt": np.ascontiguousarray(g_e.reshape(n_pad // P, P).T),
            }
        )

    res = bass_utils.run_bass_kernel_spmd(
        nc, in_maps, core_ids=list(range(E)), trace=_trace
    )

    buf = np.zeros((2 * T, D), dtype=np.float32)
    for e in range(E):
        ce = int(counts[e])
        sel = order[offsets[e] : offsets[e] + ce]
        buf[sel] = res.results[e]["y"][:ce]
    out = buf[0::2] + buf[1::2]
    if _trace:
        return out.reshape(B, S, D), res
    return out.reshape(B, S, D)


# revision 3
# speedup vs baseline: 1.0145x; 1.0145x over previous
"""MoE (top-2 of 8 experts) Trainium2 kernel — expert-parallel across 8 NeuronCores.

Full-input contract: kernel(**inputs) takes the unsharded numpy inputs and
returns the full [B, S, D] output.

Strategy:
  * Host: router (logits -> top-2 -> softmax gates), all-to-all dispatch by
    expert id (gather the tokens routed to each expert, pad to a static
    count), and the final combine (scatter-add of the two gated expert
    outputs per token).
  * Device (one expert per core): y = g * (relu(x @ W1 + b1) @ W2 + b2)
    for that expert's dispatched tokens.  Matmuls run in float32r
    (TF32-like, full PE rate); accumulation is fp32 in PSUM.
    W2 stays SBUF-resident for the whole kernel (its load is interleaved
    into block 0 so the PE isn't starved at startup); W1 is streamed once
    per 384-token block. b2 is applied with a rank-1 (K=1) matmul opening
    each PSUM accumulation group, and the gate scale rides the PSUM->SBUF
    copy (alternating scalar/vector engines).
  * W1 and the dispatched tokens are pre-arranged on the host so each DMA
    descriptor is a fat contiguous chunk per partition (4-12 KiB).
"""

import numpy as np

import concourse.tile as tile
import concourse.mybir as mybir
from concourse import bacc, bass_utils

B, S, D, F, E, TOPK = 4, 2048, 1024, 4096, 8, 2
T = B * S
P = 128
T_BLK = 384  # tokens per weight pass; 3 token tiles * 2 D-halves = 6 PSUM banks
FT = F // P  # 32 f tiles
DT = D // P  # 8 d tiles
NT = T_BLK // P  # 3 token tiles per block
DH = D // 512  # 2 output halves
F32 = mybir.dt.float32
F32R = mybir.dt.float32r
AF = mybir.ActivationFunctionType

_CACHE: dict[int, object] = {}


def _build(n_pad: int):
    """Build + compile the per-core Bass program for n_pad dispatched tokens."""
    nb = n_pad // T_BLK
    nc = bacc.Bacc("TRN2", target_bir_lowering=False, debug=False)

    # host-prearranged layouts: one fat contiguous chunk per partition
    xT = nc.dram_tensor("xT", (P, nb, DT, T_BLK), F32R, kind="ExternalInput")
    w1 = nc.dram_tensor("w1", (P, FT, DT, P), F32R, kind="ExternalInput")
    b1c = nc.dram_tensor("b1c", (P, FT), F32, kind="ExternalInput")
    w2 = nc.dram_tensor("w2", (F, D), F32R, kind="ExternalInput")
    b2r = nc.dram_tensor("b2r", (D,), F32R, kind="ExternalInput")
    gt = nc.dram_tensor("gt", (P, n_pad // P), F32, kind="ExternalInput")
    y = nc.dram_tensor("y", (n_pad, D), F32, kind="ExternalOutput")

    w2_t = w2.rearrange("(o p) d -> p o d", p=P)  # [128, 32, 1024]

    with tile.TileContext(nc) as tc:
        with (
            tc.tile_pool(name="w2p", bufs=FT) as w2p,
            tc.tile_pool(name="const", bufs=1) as constp,
            tc.tile_pool(name="xp", bufs=2) as xp,
            tc.tile_pool(name="w1p", bufs=4) as w1p,
            tc.tile_pool(name="hp", bufs=4) as hp,
            tc.tile_pool(name="op", bufs=6) as op,
            tc.tile_pool(name="ph", bufs=2, space="PSUM") as php,
            tc.tile_pool(name="py", bufs=6, space="PSUM") as pyp,
        ):
            # ---- constants ----
            b1_sb = constp.tile([P, FT], F32)
            nc.sync.dma_start(b1_sb[:], b1c[:])
            g_sb = constp.tile([P, n_pad // P], F32)
            nc.sync.dma_start(g_sb[:], gt[:])
            b2_sb = constp.tile([1, D], F32R)
            nc.sync.dma_start(b2_sb[:], b2r[None, :])
            ones_f = constp.tile([1, P], F32)
            ones_sb = constp.tile([1, P], F32R)
            nc.vector.memset(ones_f[:], 1.0)
            nc.vector.tensor_copy(ones_sb[:], ones_f[:])

            # w2 stays resident; each 512KiB chunk is loaded inside block 0's
            # f-loop, just ahead of its first use, so it doesn't starve the PE.
            w2_sb = [None] * FT

            for blk in range(nb):
                x_sb = xp.tile([P, DT, T_BLK], F32R)
                nc.sync.dma_start(x_sb[:], xT[:, blk])

                # open the 6 accumulation groups with the rank-1 b2 matmul
                psum_y = [pyp.tile([P, 512], F32, name="py") for _ in range(NT * DH)]
                for t in range(NT):
                    for dh in range(DH):
                        nc.tensor.matmul(
                            psum_y[t * DH + dh][:],
                            ones_sb[:],
                            b2_sb[:, dh * 512 : (dh + 1) * 512],
                            start=True,
                            stop=False,
                        )

                for f in range(FT):
                    w1_sb = w1p.tile([P, DT, P], F32R)
                    nc.sync.dma_start(w1_sb[:], w1[:, f])
                    if blk == 0:
                        w2f = w2p.tile([P, D], F32R, name="w2sb")
                        nc.sync.dma_start(w2f[:], w2_t[:, f])
                        w2_sb[f] = w2f
                    ph = php.tile([P, T_BLK], F32)
                    for d in range(DT):
                        nc.tensor.matmul(
                            ph[:],
                            w1_sb[:, d],
                            x_sb[:, d],
                            start=(d == 0),
                            stop=(d == DT - 1),
                        )
                    ht = hp.tile([P, T_BLK], F32R)
                    nc.scalar.activation(
                        ht[:], ph[:], AF.Relu, bias=b1_sb[:, f : f + 1], scale=1.0
                    )
                    for t in range(NT):
                        for dh in range(DH):
                            nc.tensor.matmul(
                                psum_y[t * DH + dh][:],
                                ht[:, t * P : (t + 1) * P],
                                w2_sb[f][:, dh * 512 : (dh + 1) * 512],
                                start=False,
                                stop=(f == FT - 1),
                            )

                for t in range(NT):
                    col = blk * NT + t
                    for dh in range(DH):
                        pj = psum_y[t * DH + dh]
                        ot = op.tile([P, 512], F32)
                        if (t * DH + dh) % 2 == 0:
                            nc.scalar.activation(
                                ot[:], pj[:], AF.Copy, scale=g_sb[:, col : col + 1]
                            )
                        else:
                            nc.vector.tensor_scalar_mul(
                                ot[:], pj[:], g_sb[:, col : col + 1]
                            )
                        nc.sync.dma_start(
                            y[
                                blk * T_BLK + t * P : blk * T_BLK + (t + 1) * P,
                                dh * 512 : (dh + 1) * 512,
                            ],
                            ot[:],
                        )
    nc.compile()
    return nc


def _route(x_flat, Wg, bg):
    """Top-2 routing. Returns (order, counts, offsets, gates_flat, n_pad)."""
    logits = x_flat @ Wg + bg  # [T, E]
    i1 = np.argmax(logits, axis=1)
    v1 = logits[np.arange(T), i1]
    masked = logits.copy()
    masked[np.arange(T), i1] = -np.inf
    i2 = np.argmax(masked, axis=1)
    v2 = masked[np.arange(T), i2]
    # softmax over the two selected logits
    e2 = np.exp(v2 - v1)
    g1 = 1.0 / (1.0 + e2)
    g2 = e2 / (1.0 + e2)
    eid = np.stack([i1, i2], 1).reshape(-1)  # [2T]
    gates = np.stack([g1, g2], 1).reshape(-1).astype(np.float32)
    order = np.argsort(eid, kind="stable")
    counts = np.bincount(eid, minlength=E)
    offsets = np.concatenate([[0], np.cumsum(counts)])
    n_pad = max(T_BLK, int(-(-counts.max() // T_BLK)) * T_BLK)
    return order, counts, offsets, gates, n_pad


def kernel(x, Wg, bg, W1, b1, W2, b2, _trace=False):
    x = np.ascontiguousarray(np.asarray(x, dtype=np.float32))
    Wg = np.asarray(Wg, dtype=np.float32)
    bg = np.asarray(bg, dtype=np.float32)
    W1 = np.asarray(W1, dtype=np.float32)
    b1 = np.asarray(b1, dtype=np.float32)
    W2 = np.asarray(W2, dtype=np.float32)
    b2 = np.asarray(b2, dtype=np.float32)

    x_flat = x.reshape(T, D)
    order, counts, offsets, gates, n_pad = _route(x_flat, Wg, bg)
    nb = n_pad // T_BLK

    if n_pad not in _CACHE:
        _CACHE[n_pad] = _build(n_pad)
    nc = _CACHE[n_pad]

    in_maps = []
    for e in range(E):
        ce = int(counts[e])
        sel = order[offsets[e] : offsets[e] + ce]
        toks = sel >> 1
        xd = np.zeros((n_pad, D), dtype=np.float32)
        xd[:ce] = x_flat[toks]
        # [n, d] -> [p, blk, o, t] with n = blk*T_BLK + t, d = o*P + p
        xT_e = np.ascontiguousarray(
            xd.reshape(nb, T_BLK, DT, P).transpose(3, 0, 2, 1)
        )
        # [d, f] -> [p, ft, o, m] with d = o*P + p, f = ft*P + m
        w1_e = np.ascontiguousarray(
            W1[e].reshape(DT, P, FT, P).transpose(1, 2, 0, 3)
        )
        g_e = np.zeros(n_pad, dtype=np.float32)
        g_e[:ce] = gates[sel]
        in_maps.append(
            {
                "xT": xT_e,
                "w1": w1_e,
                "b1c": np.ascontiguousarray(b1[e].reshape(FT, P).T),
                "w2": np.ascontiguousarray(W2[e]),
                "b2r": np.ascontiguousarray(b2[e]),
                "gt": np.ascontiguousarray(g_e.reshape(n_pad // P, P).T),
            }
        )

    res = bass_utils.run_bass_kernel_spmd(
        nc, in_maps, core_ids=list(range(E)), trace=_trace
    )

    buf = np.zeros((2 * T, D), dtype=np.float32)
    for e in range(E):
        ce = int(counts[e])
        sel = order[offsets[e] : offsets[e] + ce]
        buf[sel] = res.results[e]["y"][:ce]
    out = buf[0::2] + buf[1::2]
    if _trace:
        return out.reshape(B, S, D), res
    return out.reshape(B, S, D)


# revision 5
# speedup vs baseline: 1.0416x; 1.0268x over previous
"""MoE (top-2 of 8 experts) Trainium2 kernel — expert-parallel across 8 NeuronCores.

Full-input contract: kernel(**inputs) takes the unsharded numpy inputs and
returns the full [B, S, D] output.

Strategy:
  * Host: router (logits -> top-2 -> softmax gates), all-to-all dispatch by
    expert id (gather the tokens routed to each expert, pad to a static
    count), and the final combine (scatter-add of the two gated expert
    outputs per token, plus the gated b2 term).
  * Device (one expert per core): y = g * (relu(x @ W1 + b1) @ W2) for that
    expert's dispatched tokens.  Matmuls run in float32r (TF32-like, full
    PE rate); accumulation is fp32 in PSUM.  W2 stays SBUF-resident for
    the whole kernel (its load is interleaved into block 0 so the PE isn't
    starved at startup); W1 is streamed once per token block.  The gate
    scale rides the PSUM->SBUF copy (alternating scalar/vector engines).
  * Tokens are processed in blocks of 384 (3 token tiles x 2 D-halves = 6
    PSUM banks accumulate the second matmul over all 32 F-tiles) with an
    optional 256-token tail block, so the padded count is a multiple of
    128, not 384.
  * W1 and the dispatched tokens are pre-arranged on the host so each DMA
    descriptor is a fat contiguous chunk per partition (4 KiB).
"""

import numpy as np

import concourse.tile as tile
import concourse.mybir as mybir
from concourse import bacc, bass_utils

B, S, D, F, E, TOPK = 4, 2048, 1024, 4096, 8, 2
T = B * S
P = 128
FT = F // P  # 32 f tiles
DT = D // P  # 8 d tiles
DH = D // 512  # 2 output halves
F32 = mybir.dt.float32
F32R = mybir.dt.float32r
AF = mybir.ActivationFunctionType

_CACHE: dict[tuple, object] = {}


def _block_sizes(n_pad: int) -> list[int]:
    """Decompose n_pad (multiple of 128, >=256) into blocks of 384 and 256."""
    rem = n_pad % 384
    if rem == 0:
        return [384] * (n_pad // 384)
    if rem == 128:
        assert n_pad >= 512
        return [384] * (n_pad // 384 - 1) + [256, 256]
    return [384] * (n_pad // 384) + [256]


def _build(n_pad: int):
    """Build + compile the per-core Bass program for n_pad dispatched tokens."""
    sizes = _block_sizes(n_pad)
    nc = bacc.Bacc("TRN2", target_bir_lowering=False, debug=False)

    # host-prearranged layouts: one fat contiguous chunk per partition
    xT = nc.dram_tensor("xT", (P, DT, n_pad), F32R, kind="ExternalInput")
    w1 = nc.dram_tensor("w1", (P, FT, DT, P), F32R, kind="ExternalInput")
    b1c = nc.dram_tensor("b1c", (P, FT), F32, kind="ExternalInput")
    w2 = nc.dram_tensor("w2", (F, D), F32R, kind="ExternalInput")
    gt = nc.dram_tensor("gt", (P, n_pad // P), F32, kind="ExternalInput")
    y = nc.dram_tensor("y", (n_pad, D), F32, kind="ExternalOutput")

    w2_t = w2.rearrange("(o p) d -> p o d", p=P)  # [128, 32, 1024]

    with tile.TileContext(nc) as tc:
        with (
            tc.tile_pool(name="w2p", bufs=FT) as w2p,
            tc.tile_pool(name="const", bufs=1) as constp,
            tc.tile_pool(name="xp", bufs=2 * DT) as xp,
            tc.tile_pool(name="w1p", bufs=4) as w1p,
            tc.tile_pool(name="hp", bufs=4) as hp,
            tc.tile_pool(name="op", bufs=6) as op,
            tc.tile_pool(name="ph", bufs=2, space="PSUM") as php,
            tc.tile_pool(name="py", bufs=6, space="PSUM") as pyp,
        ):
            # ---- constants ----
            b1_sb = constp.tile([P, FT], F32)
            nc.sync.dma_start(b1_sb[:], b1c[:])
            g_sb = constp.tile([P, n_pad // P], F32)
            nc.sync.dma_start(g_sb[:], gt[:])

            # w2 stays resident; each 512KiB chunk is loaded inside block 0's
            # f-loop, just ahead of its first use, so it doesn't starve the PE.
            w2_sb = [None] * FT

            tok = 0
            for blk, tb in enumerate(sizes):
                nt = tb // P
                x_sb = [xp.tile([P, 512], F32R, name="xsb") for _ in range(DT)]
                for d in range(DT):
                    nc.sync.dma_start(
                        x_sb[d][:, :tb], xT[:, d, tok : tok + tb]
                    )

                psum_y = [
                    pyp.tile([P, 512], F32, name="py") for _ in range(nt * DH)
                ]

                for f in range(FT):
                    w1_sb = w1p.tile([P, DT, P], F32R)
                    nc.sync.dma_start(w1_sb[:], w1[:, f])
                    if blk == 0:
                        w2f = w2p.tile([P, D], F32R, name="w2sb")
                        nc.sync.dma_start(w2f[:], w2_t[:, f])
                        w2_sb[f] = w2f
                    ph = php.tile([P, 512], F32, name="ph")
                    for d in range(DT):
                        nc.tensor.matmul(
                            ph[:, :tb],
                            w1_sb[:, d],
                            x_sb[d][:, :tb],
                            start=(d == 0),
                            stop=(d == DT - 1),
                        )
                    ht = hp.tile([P, 512], F32R, name="ht")
                    nc.scalar.activation(
                        ht[:, :tb], ph[:, :tb], AF.Relu,
                        bias=b1_sb[:, f : f + 1], scale=1.0,
                    )
                    for t in range(nt):
                        for dh in range(DH):
                            nc.tensor.matmul(
                                psum_y[t * DH + dh][:],
                                ht[:, t * P : (t + 1) * P],
                                w2_sb[f][:, dh * 512 : (dh + 1) * 512],
                                start=(f == 0),
                                stop=(f == FT - 1),
                            )

                for t in range(nt):
                    col = tok // P + t
                    for dh in range(DH):
                        pj = psum_y[t * DH + dh]
                        ot = op.tile([P, 512], F32)
                        if (t * DH + dh) % 2 == 0:
                            nc.scalar.activation(
                                ot[:], pj[:], AF.Copy, scale=g_sb[:, col : col + 1]
                            )
                        else:
                            nc.vector.tensor_scalar_mul(
                                ot[:], pj[:], g_sb[:, col : col + 1]
                            )
                        nc.sync.dma_start(
                            y[
                                tok + t * P : tok + (t + 1) * P,
                                dh * 512 : (dh + 1) * 512,
                            ],
                            ot[:],
                        )
                tok += tb
    nc.compile()
    return nc


def _route(x_flat, Wg, bg):
    """Top-2 routing. Returns (order, counts, offsets, pair gate/idx arrays, n_pad)."""
    logits = x_flat @ Wg + bg  # [T, E]
    i1 = np.argmax(logits, axis=1)
    v1 = logits[np.arange(T), i1]
    masked = logits.copy()
    masked[np.arange(T), i1] = -np.inf
    i2 = np.argmax(masked, axis=1)
    v2 = masked[np.arange(T), i2]
    # softmax over the two selected logits
    e2 = np.exp(v2 - v1)
    g1 = 1.0 / (1.0 + e2)
    g2 = e2 / (1.0 + e2)
    eid = np.stack([i1, i2], 1).reshape(-1)  # [2T]
    gates = np.stack([g1, g2], 1).reshape(-1).astype(np.float32)
    order = np.argsort(eid, kind="stable")
    counts = np.bincount(eid, minlength=E)
    offsets = np.concatenate([[0], np.cumsum(counts)])
    n_pad = max(256, int(-(-counts.max() // P)) * P)
    return order, counts, offsets, gates, n_pad


def kernel(x, Wg, bg, W1, b1, W2, b2, _trace=False):
    x = np.ascontiguousarray(np.asarray(x, dtype=np.float32))
    Wg = np.asarray(Wg, dtype=np.float32)
    bg = np.asarray(bg, dtype=np.float32)
    W1 = np.asarray(W1, dtype=np.float32)
    b1 = np.asarray(b1, dtype=np.float32)
    W2 = np.asarray(W2, dtype=np.float32)
    b2 = np.asarray(b2, dtype=np.float32)

    x_flat = x.reshape(T, D)
    order, counts, offsets, gates, n_pad = _route(x_flat, Wg, bg)

    if n_pad not in _CACHE:
        _CACHE[n_pad] = _build(n_pad)
    nc = _CACHE[n_pad]

    in_maps = []
    for e in range(E):
        ce = int(counts[e])
        sel = order[offsets[e] : offsets[e] + ce]
        toks = sel >> 1
        xd = np.zeros((n_pad, D), dtype=np.float32)
        xd[:ce] = x_flat[toks]
        # [n, d] -> [p, o, n] with d = o*P + p
        xT_e = np.ascontiguousarray(xd.reshape(n_pad, DT, P).transpose(2, 1, 0))
        # [d, f] -> [p, ft, o, m] with d = o*P + p, f = ft*P + m
        w1_e = np.ascontiguousarray(
            W1[e].reshape(DT, P, FT, P).transpose(1, 2, 0, 3)
        )
        g_e = np.zeros(n_pad, dtype=np.float32)
        g_e[:ce] = gates[sel]
        in_maps.append(
            {
                "xT": xT_e,
                "w1": w1_e,
                "b1c": np.ascontiguousarray(b1[e].reshape(FT, P).T),
                "w2": np.ascontiguousarray(W2[e]),
                "gt": np.ascontiguousarray(g_e.reshape(n_pad // P, P).T),
            }
        )

    res = bass_utils.run_bass_kernel_spmd(
        nc, in_maps, core_ids=list(range(E)), trace=_trace
    )

    buf = np.zeros((2 * T, D), dtype=np.float32)
    for e in range(E):
        ce = int(counts[e])
        sel = order[offsets[e] : offsets[e] + ce]
        buf[sel] = res.results[e]["y"][:ce]
    out = buf[0::2] + buf[1::2]
    # b2 is applied host-side: out_t += g1*b2[e1] + g2*b2[e2]
    g_pairs = gates.reshape(T, 2)
    # recover expert ids per pair from the order/offsets partition
    eid_flat = np.empty(2 * T, dtype=np.int64)
    for e in range(E):
        eid_flat[order[offsets[e] : offsets[e + 1]]] = e
    i_pairs = eid_flat.reshape(T, 2)
    out += g_pairs[:, 0:1] * b2[i_pairs[:, 0]] + g_pairs[:, 1:2] * b2[i_pairs[:, 1]]
    if _trace:
        return out.reshape(B, S, D), res
    return out.reshape(B, S, D)


# revision 6
# speedup vs baseline: 1.0594x; 1.0170x over previous
"""MoE (top-2 of 8 experts) Trainium2 kernel — expert-parallel across 8 NeuronCores.

Full-input contract: kernel(**inputs) takes the unsharded numpy inputs and
returns the full [B, S, D] output.

Strategy:
  * Host: router (logits -> top-2 -> softmax gates), all-to-all dispatch by
    expert id (gather the tokens routed to each expert, pad to a static
    count), and the final combine (scatter-add of the two gated expert
    outputs per token, plus the gated b2 term).
  * Device (one expert per core): y = g * (relu(x @ W1 + b1) @ W2) for that
    expert's dispatched tokens.  Matmuls run in float32r (TF32-like, full
    PE rate); accumulation is fp32 in PSUM.  W2 stays SBUF-resident for
    the whole kernel (its load is interleaved into block 0 so the PE isn't
    starved at startup); W1 is streamed once per token block.  The gate
    scale rides the PSUM->SBUF copy (alternating scalar/vector engines).
  * Tokens are processed in blocks of 384 (3 token tiles x 2 D-halves = 6
    PSUM banks accumulate the second matmul over all 32 F-tiles) with an
    optional 256-token tail block, so the padded count is a multiple of
    128, not 384.
  * W1 and the dispatched tokens are pre-arranged on the host so each DMA
    descriptor is a fat contiguous chunk per partition (4 KiB).
"""

import numpy as np

import concourse.tile as tile
import concourse.mybir as mybir
from concourse import bacc, bass_utils

B, S, D, F, E, TOPK = 4, 2048, 1024, 4096, 8, 2
T = B * S
P = 128
FT = F // P  # 32 f tiles
DT = D // P  # 8 d tiles
DH = D // 512  # 2 output halves
F32 = mybir.dt.float32
F32R = mybir.dt.float32r
AF = mybir.ActivationFunctionType

_CACHE: dict[tuple, object] = {}


def _block_sizes(n_pad: int) -> list[int]:
    """Decompose n_pad (multiple of 128, >=256) into blocks of 384 and 256."""
    rem = n_pad % 384
    if rem == 0:
        return [384] * (n_pad // 384)
    if rem == 128:
        assert n_pad >= 512
        return [384] * (n_pad // 384 - 1) + [256, 256]
    return [384] * (n_pad // 384) + [256]


def _build(n_pad: int):
    """Build + compile the per-core Bass program for n_pad dispatched tokens."""
    sizes = _block_sizes(n_pad)
    nc = bacc.Bacc("TRN2", target_bir_lowering=False, debug=False)

    # host-prearranged layouts: one fat contiguous chunk per partition
    xT = nc.dram_tensor("xT", (P, DT, n_pad), F32R, kind="ExternalInput")
    w1 = nc.dram_tensor("w1", (P, FT, DT, P), F32R, kind="ExternalInput")
    b1c = nc.dram_tensor("b1c", (P, FT), F32, kind="ExternalInput")
    w2 = nc.dram_tensor("w2", (F, D), F32R, kind="ExternalInput")
    gt = nc.dram_tensor("gt", (P, n_pad // P), F32, kind="ExternalInput")
    y = nc.dram_tensor("y", (n_pad, D), F32, kind="ExternalOutput")

    w2_t = w2.rearrange("(o p) d -> p o d", p=P)  # [128, 32, 1024]

    PREF = 2  # next-block w1 tiles prefetched during the previous block

    with tile.TileContext(nc) as tc:
        with (
            tc.tile_pool(name="w2p", bufs=FT) as w2p,
            tc.tile_pool(name="const", bufs=1) as constp,
            tc.tile_pool(name="xp", bufs=2 * DT) as xp,
            tc.tile_pool(name="w1p", bufs=6) as w1p,
            tc.tile_pool(name="hp", bufs=4) as hp,
            tc.tile_pool(name="op", bufs=4) as op,
            tc.tile_pool(name="ph", bufs=2, space="PSUM") as php,
            tc.tile_pool(name="py", bufs=6, space="PSUM") as pyp,
        ):
            # ---- constants ----
            b1_sb = constp.tile([P, FT], F32)
            nc.sync.dma_start(b1_sb[:], b1c[:])
            g_sb = constp.tile([P, n_pad // P], F32)
            nc.sync.dma_start(g_sb[:], gt[:])

            # w2 stays resident; each 512KiB chunk is loaded inside block 0's
            # f-loop, just ahead of its first use, so it doesn't starve the PE.
            w2_sb = [None] * FT

            def emit_x(tok, tb):
                xs = [xp.tile([P, 512], F32R, name="xsb") for _ in range(DT)]
                for d in range(DT):
                    nc.sync.dma_start(xs[d][:, :tb], xT[:, d, tok : tok + tb])
                return xs

            def emit_w1(f):
                t = w1p.tile([P, DT, P], F32R)
                h = DT // 2
                nc.sync.dma_start(t[:, :h], w1[:, f, :h])
                nc.sync.dma_start(t[:, h:], w1[:, f, h:])
                return t

            x_cur = emit_x(0, sizes[0])
            w1_pref = [emit_w1(f) for f in range(PREF)]

            tok = 0
            for blk, tb in enumerate(sizes):
                nt = tb // P
                x_sb = x_cur
                w1_cur, w1_pref = w1_pref, []

                psum_y = [
                    pyp.tile([P, 512], F32, name="py") for _ in range(nt * DH)
                ]

                for f in range(FT):
                    w1_sb = w1_cur.pop(0) if w1_cur else emit_w1(f)
                    if blk == 0:
                        w2f = w2p.tile([P, D], F32R, name="w2sb")
                        nc.sync.dma_start(w2f[:], w2_t[:, f])
                        w2_sb[f] = w2f
                    if blk + 1 < len(sizes):
                        if f == FT - 10:
                            x_cur = emit_x(tok + tb, sizes[blk + 1])
                        elif f >= FT - PREF:
                            w1_pref.append(emit_w1(f - (FT - PREF)))
                    ph = php.tile([P, 512], F32, name="ph")
                    for d in range(DT):
                        nc.tensor.matmul(
                            ph[:, :tb],
                            w1_sb[:, d],
                            x_sb[d][:, :tb],
                            start=(d == 0),
                            stop=(d == DT - 1),
                        )
                    ht = hp.tile([P, 512], F32R, name="ht")
                    nc.scalar.activation(
                        ht[:, :tb], ph[:, :tb], AF.Relu,
                        bias=b1_sb[:, f : f + 1], scale=1.0,
                    )
                    for t in range(nt):
                        for dh in range(DH):
                            nc.tensor.matmul(
                                psum_y[t * DH + dh][:],
                                ht[:, t * P : (t + 1) * P],
                                w2_sb[f][:, dh * 512 : (dh + 1) * 512],
                                start=(f == 0),
                                stop=(f == FT - 1),
                            )

                for t in range(nt):
                    col = tok // P + t
                    for dh in range(DH):
                        pj = psum_y[t * DH + dh]
                        ot = op.tile([P, 512], F32)
                        if (t * DH + dh) % 2 == 0:
                            nc.scalar.activation(
                                ot[:], pj[:], AF.Copy, scale=g_sb[:, col : col + 1]
                            )
                        else:
                            nc.vector.tensor_scalar_mul(
                                ot[:], pj[:], g_sb[:, col : col + 1]
                            )
                        nc.sync.dma_start(
                            y[
                                tok + t * P : tok + (t + 1) * P,
                                dh * 512 : (dh + 1) * 512,
                            ],
                            ot[:],
                        )
                tok += tb
    nc.compile()
    return nc


def _route(x_flat, Wg, bg):
    """Top-2 routing. Returns (order, counts, offsets, pair gate/idx arrays, n_pad)."""
    logits = x_flat @ Wg + bg  # [T, E]
    i1 = np.argmax(logits, axis=1)
    v1 = logits[np.arange(T), i1]
    masked = logits.copy()
    masked[np.arange(T), i1] = -np.inf
    i2 = np.argmax(masked, axis=1)
    v2 = masked[np.arange(T), i2]
    # softmax over the two selected logits
    e2 = np.exp(v2 - v1)
    g1 = 1.0 / (1.0 + e2)
    g2 = e2 / (1.0 + e2)
    eid = np.stack([i1, i2], 1).reshape(-1)  # [2T]
    gates = np.stack([g1, g2], 1).reshape(-1).astype(np.float32)
    order = np.argsort(eid, kind="stable")
    counts = np.bincount(eid, minlength=E)
    offsets = np.concatenate([[0], np.cumsum(counts)])
    n_pad = max(256, int(-(-counts.max() // P)) * P)
    return order, counts, offsets, gates, n_pad


def kernel(x, Wg, bg, W1, b1, W2, b2, _trace=False):
    x = np.ascontiguousarray(np.asarray(x, dtype=np.float32))
    Wg = np.asarray(Wg, dtype=np.float32)
    bg = np.asarray(bg, dtype=np.float32)
    W1 = np.asarray(W1, dtype=np.float32)
    b1 = np.asarray(b1, dtype=np.float32)
    W2 = np.asarray(W2, dtype=np.float32)
    b2 = np.asarray(b2, dtype=np.float32)

    x_flat = x.reshape(T, D)
    order, counts, offsets, gates, n_pad = _route(x_flat, Wg, bg)

    if n_pad not in _CACHE:
        _CACHE[n_pad] = _build(n_pad)
    nc = _CACHE[n_pad]

    in_maps = []
    for e in range(E):
        ce = int(counts[e])
        sel = order[offsets[e] : offsets[e] + ce]
        toks = sel >> 1
        xd = np.zeros((n_pad, D), dtype=np.float32)
        xd[:ce] = x_flat[toks]
        # [n, d] -> [p, o, n] with d = o*P + p
        xT_e = np.ascontiguousarray(xd.reshape(n_pad, DT, P).transpose(2, 1, 0))
        # [d, f] -> [p, ft, o, m] with d = o*P + p, f = ft*P + m
        w1_e = np.ascontiguousarray(
            W1[e].reshape(DT, P, FT, P).transpose(1, 2, 0, 3)
        )
        g_e = np.zeros(n_pad, dtype=np.float32)
        g_e[:ce] = gates[sel]
        in_maps.append(
            {
                "xT": xT_e,
                "w1": w1_e,
                "b1c": np.ascontiguousarray(b1[e].reshape(FT, P).T),
                "w2": np.ascontiguousarray(W2[e]),
                "gt": np.ascontiguousarray(g_e.reshape(n_pad // P, P).T),
            }
        )

    res = bass_utils.run_bass_kernel_spmd(
        nc, in_maps, core_ids=list(range(E)), trace=_trace
    )

    buf = np.zeros((2 * T, D), dtype=np.float32)
    for e in range(E):
        ce = int(counts[e])
        sel = order[offsets[e] : offsets[e] + ce]
        buf[sel] = res.results[e]["y"][:ce]
    out = buf[0::2] + buf[1::2]
    # b2 is applied host-side: out_t += g1*b2[e1] + g2*b2[e2]
    g_pairs = gates.reshape(T, 2)
    # recover expert ids per pair from the order/offsets partition
    eid_flat = np.empty(2 * T, dtype=np.int64)
    for e in range(E):
        eid_flat[order[offsets[e] : offsets[e + 1]]] = e
    i_pairs = eid_flat.reshape(T, 2)
    out += g_pairs[:, 0:1] * b2[i_pairs[:, 0]] + g_pairs[:, 1:2] * b2[i_pairs[:, 1]]
    if _trace:
        return out.reshape(B, S, D), res
    return out.reshape(B, S, D)


# revision 8
# speedup vs baseline: 1.0632x; 1.0036x over previous
"""MoE (top-2 of 8 experts) Trainium2 kernel — expert-parallel across 8 NeuronCores.

Full-input contract: kernel(**inputs) takes the unsharded numpy inputs and
returns the full [B, S, D] output.

Strategy:
  * Host: router (logits -> top-2 -> softmax gates), all-to-all dispatch by
    expert id (gather the tokens routed to each expert, pad to a static
    count), and the final combine (scatter-add of the two gated expert
    outputs per token, plus the gated b2 term).
  * Device (one expert per core): y = g * (relu(x @ W1 + b1) @ W2) for that
    expert's dispatched tokens.  Matmuls run in float32r (TF32-like, full
    PE rate); accumulation is fp32 in PSUM.  W2 stays SBUF-resident for
    the whole kernel (its load is interleaved into block 0 so the PE isn't
    starved at startup); W1 is streamed once per token block.  The gate
    scale rides the PSUM->SBUF copy (alternating scalar/vector engines).
  * Tokens are processed in blocks of 384 (3 token tiles x 2 D-halves = 6
    PSUM banks accumulate the second matmul over all 32 F-tiles) with an
    optional 256-token tail block, so the padded count is a multiple of
    128, not 384.
  * W1 and the dispatched tokens are pre-arranged on the host so each DMA
    descriptor is a fat contiguous chunk per partition (4 KiB).
"""

import numpy as np

import concourse.tile as tile
import concourse.mybir as mybir
from concourse import bacc, bass_utils

B, S, D, F, E, TOPK = 4, 2048, 1024, 4096, 8, 2
T = B * S
P = 128
FT = F // P  # 32 f tiles
DT = D // P  # 8 d tiles
DH = D // 512  # 2 output halves
F32 = mybir.dt.float32
F32R = mybir.dt.float32r
AF = mybir.ActivationFunctionType

_CACHE: dict[tuple, object] = {}


def _block_sizes(n_pad: int) -> list[int]:
    """Decompose n_pad (multiple of 128, >=256) into blocks of 384 and 256."""
    rem = n_pad % 384
    if rem == 0:
        return [384] * (n_pad // 384)
    if rem == 128:
        assert n_pad >= 512
        return [384] * (n_pad // 384 - 1) + [256, 256]
    return [384] * (n_pad // 384) + [256]


def _build(n_pad: int):
    """Build + compile the per-core Bass program for n_pad dispatched tokens."""
    sizes = _block_sizes(n_pad)
    nc = bacc.Bacc("TRN2", target_bir_lowering=False, debug=False)

    # host-prearranged layouts: one fat contiguous chunk per partition
    xT = nc.dram_tensor("xT", (P, DT, n_pad), F32R, kind="ExternalInput")
    w1 = nc.dram_tensor("w1", (P, FT, DT, P), F32R, kind="ExternalInput")
    b1c = nc.dram_tensor("b1c", (P, FT), F32, kind="ExternalInput")
    w2 = nc.dram_tensor("w2", (F, D), F32R, kind="ExternalInput")
    gt = nc.dram_tensor("gt", (P, n_pad // P), F32, kind="ExternalInput")
    y = nc.dram_tensor("y", (n_pad, D), F32, kind="ExternalOutput")

    w2_t = w2.rearrange("(o p) d -> p o d", p=P)  # [128, 32, 1024]

    PREF = 2  # next-block w1 tiles prefetched during the previous block

    with tile.TileContext(nc) as tc:
        with (
            tc.tile_pool(name="w2p", bufs=FT) as w2p,
            tc.tile_pool(name="const", bufs=1) as constp,
            tc.tile_pool(name="xp", bufs=2 * DT) as xp,
            tc.tile_pool(name="w1p", bufs=6) as w1p,
            tc.tile_pool(name="hp", bufs=4) as hp,
            tc.tile_pool(name="op", bufs=4) as op,
            tc.tile_pool(name="ph", bufs=2, space="PSUM") as php,
            tc.tile_pool(name="py", bufs=6, space="PSUM") as pyp,
        ):
            # w2 stays resident; each 512KiB chunk is loaded inside block 0's
            # f-loop, a few iterations ahead of its first use, so it doesn't
            # starve the PE.
            w2_sb = [None] * FT
            W2_AHEAD = 3

            def emit_x(tok, tb, first=0):
                xs = [xp.tile([P, 512], F32R, name="xsb") for _ in range(DT)]
                for d in list(range(first, DT)) + list(range(first)):
                    nc.sync.dma_start(xs[d][:, :tb], xT[:, d, tok : tok + tb])
                return xs

            def emit_w1(f):
                t = w1p.tile([P, DT, P], F32R)
                h = DT // 2
                nc.sync.dma_start(t[:, :h], w1[:, f, :h])
                nc.sync.dma_start(t[:, h:], w1[:, f, h:])
                return t

            def emit_w2(f):
                w2f = w2p.tile([P, D], F32R, name="w2sb")
                nc.sync.dma_start(w2f[:], w2_t[:, f])
                w2_sb[f] = w2f

            # prologue: critical-path DMAs first (x d=0, w1 f=0), then the rest
            xs0 = [xp.tile([P, 512], F32R, name="xsb") for _ in range(DT)]
            nc.sync.dma_start(xs0[0][:, : sizes[0]], xT[:, 0, : sizes[0]])
            w1_pref = [emit_w1(0)]
            for d in range(1, DT):
                nc.sync.dma_start(xs0[d][:, : sizes[0]], xT[:, d, : sizes[0]])
            x_cur = xs0
            w1_pref.append(emit_w1(1))
            b1_sb = constp.tile([P, FT], F32)
            nc.sync.dma_start(b1_sb[:], b1c[:])
            g_sb = constp.tile([P, n_pad // P], F32)
            nc.sync.dma_start(g_sb[:], gt[:])
            for f in range(W2_AHEAD):
                emit_w2(f)

            tok = 0
            for blk, tb in enumerate(sizes):
                nt = tb // P
                x_sb = x_cur
                w1_cur, w1_pref = w1_pref, []

                psum_y = [
                    pyp.tile([P, 512], F32, name="py") for _ in range(nt * DH)
                ]

                for f in range(FT):
                    w1_sb = w1_cur.pop(0) if w1_cur else emit_w1(f)
                    if blk == 0 and f + W2_AHEAD < FT:
                        emit_w2(f + W2_AHEAD)
                    if blk + 1 < len(sizes):
                        if f == FT - 16:
                            x_cur = emit_x(tok + tb, sizes[blk + 1])
                        elif f >= FT - PREF:
                            w1_pref.append(emit_w1(f - (FT - PREF)))
                    ph = php.tile([P, 512], F32, name="ph")
                    for d in range(DT):
                        nc.tensor.matmul(
                            ph[:, :tb],
                            w1_sb[:, d],
                            x_sb[d][:, :tb],
                            start=(d == 0),
                            stop=(d == DT - 1),
                        )
                    ht = hp.tile([P, 512], F32R, name="ht")
                    nc.scalar.activation(
                        ht[:, :tb], ph[:, :tb], AF.Relu,
                        bias=b1_sb[:, f : f + 1], scale=1.0,
                    )
                    for t in range(nt):
                        for dh in range(DH):
                            nc.tensor.matmul(
                                psum_y[t * DH + dh][:],
                                ht[:, t * P : (t + 1) * P],
                                w2_sb[f][:, dh * 512 : (dh + 1) * 512],
                                start=(f == 0),
                                stop=(f == FT - 1),
                            )

                for t in range(nt):
                    col = tok // P + t
                    for dh in range(DH):
                        pj = psum_y[t * DH + dh]
                        ot = op.tile([P, 512], F32)
                        if (t * DH + dh) % 2 == 0:
                            nc.scalar.activation(
                                ot[:], pj[:], AF.Copy, scale=g_sb[:, col : col + 1]
                            )
                        else:
                            nc.vector.tensor_scalar_mul(
                                ot[:], pj[:], g_sb[:, col : col + 1]
                            )
                        nc.sync.dma_start(
                            y[
                                tok + t * P : tok + (t + 1) * P,
                                dh * 512 : (dh + 1) * 512,
                            ],
                            ot[:],
                        )
                tok += tb
    nc.compile()
    return nc


def _route(x_flat, Wg, bg):
    """Top-2 routing. Returns (order, counts, offsets, pair gate/idx arrays, n_pad)."""
    logits = x_flat @ Wg + bg  # [T, E]
    i1 = np.argmax(logits, axis=1)
    v1 = logits[np.arange(T), i1]
    masked = logits.copy()
    masked[np.arange(T), i1] = -np.inf
    i2 = np.argmax(masked, axis=1)
    v2 = masked[np.arange(T), i2]
    # softmax over the two selected logits
    e2 = np.exp(v2 - v1)
    g1 = 1.0 / (1.0 + e2)
    g2 = e2 / (1.0 + e2)
    eid = np.stack([i1, i2], 1).reshape(-1)  # [2T]
    gates = np.stack([g1, g2], 1).reshape(-1).astype(np.float32)
    order = np.argsort(eid, kind="stable")
    counts = np.bincount(eid, minlength=E)
    offsets = np.concatenate([[0], np.cumsum(counts)])
    n_pad = max(256, int(-(-counts.max() // P)) * P)
    return order, counts, offsets, gates, n_pad


def kernel(x, Wg, bg, W1, b1, W2, b2, _trace=False):
    x = np.ascontiguousarray(np.asarray(x, dtype=np.float32))
    Wg = np.asarray(Wg, dtype=np.float32)
    bg = np.asarray(bg, dtype=np.float32)
    W1 = np.asarray(W1, dtype=np.float32)
    b1 = np.asarray(b1, dtype=np.float32)
    W2 = np.asarray(W2, dtype=np.float32)
    b2 = np.asarray(b2, dtype=np.float32)

    x_flat = x.reshape(T, D)
    order, counts, offsets, gates, n_pad = _route(x_flat, Wg, bg)

    if n_pad not in _CACHE:
        _CACHE[n_pad] = _build(n_pad)
    nc = _CACHE[n_pad]

    in_maps = []
    for e in range(E):
        ce = int(counts[e])
        sel = order[offsets[e] : offsets[e] + ce]
        toks = sel >> 1
        xd = np.zeros((n_pad, D), dtype=np.float32)
        xd[:ce] = x_flat[toks]
        # [n, d] -> [p, o, n] with d = o*P + p
        xT_e = np.ascontiguousarray(xd.reshape(n_pad, DT, P).transpose(2, 1, 0))
        # [d, f] -> [p, ft, o, m] with d = o*P + p, f = ft*P + m
        w1_e = np.ascontiguousarray(
            W1[e].reshape(DT, P, FT, P).transpose(1, 2, 0, 3)
        )
        g_e = np.zeros(n_pad, dtype=np.float32)
        g_e[:ce] = gates[sel]
        in_maps.append(
            {
                "xT": xT_e,
                "w1": w1_e,
                "b1c": np.ascontiguousarray(b1[e].reshape(FT, P).T),
                "w2": np.ascontiguousarray(W2[e]),
                "gt": np.ascontiguousarray(g_e.reshape(n_pad // P, P).T),
            }
        )

    res = bass_utils.run_bass_kernel_spmd(
        nc, in_maps, core_ids=list(range(E)), trace=_trace
    )

    buf = np.zeros((2 * T, D), dtype=np.float32)
    for e in range(E):
        ce = int(counts[e])
        sel = order[offsets[e] : offsets[e] + ce]
        buf[sel] = res.results[e]["y"][:ce]
    out = buf[0::2] + buf[1::2]
    # b2 is applied host-side: out_t += g1*b2[e1] + g2*b2[e2]
    g_pairs = gates.reshape(T, 2)
    # recover expert ids per pair from the order/offsets partition
    eid_flat = np.empty(2 * T, dtype=np.int64)
    for e in range(E):
        eid_flat[order[offsets[e] : offsets[e + 1]]] = e
    i_pairs = eid_flat.reshape(T, 2)
    out += g_pairs[:, 0:1] * b2[i_pairs[:, 0]] + g_pairs[:, 1:2] * b2[i_pairs[:, 1]]
    if _trace:
        return out.reshape(B, S, D), res
    return out.reshape(B, S, D)


# revision 9
# speedup vs baseline: 1.0910x; 1.0261x over previous
"""MoE (top-2 of 8 experts) Trainium2 kernel — expert-parallel across 8 NeuronCores.

Full-input contract: kernel(**inputs) takes the unsharded numpy inputs and
returns the full [B, S, D] output.

Strategy:
  * Host: router (logits -> top-2 -> softmax gates), all-to-all dispatch by
    expert id (gather the tokens routed to each expert, pad to a static
    count), and the final combine (scatter-add of the two gated expert
    outputs per token, plus the gated b2 term).
  * Device (one expert per core): y = g * (relu(x @ W1 + b1) @ W2) for that
    expert's dispatched tokens.  Matmuls run in float32r (TF32-like, full
    PE rate); accumulation is fp32 in PSUM.  W2 stays SBUF-resident for
    the whole kernel (its load is interleaved into block 0 so the PE isn't
    starved at startup); W1 is streamed once per token block.  The gate
    scale rides the PSUM->SBUF copy (alternating scalar/vector engines).
  * Tokens are processed in blocks of 384 (3 token tiles x 2 D-halves = 6
    PSUM banks accumulate the second matmul over all 32 F-tiles) with an
    optional 256-token tail block, so the padded count is a multiple of
    128, not 384.
  * W1 and the dispatched tokens are pre-arranged on the host so each DMA
    descriptor is a fat contiguous chunk per partition (4 KiB).
"""

import numpy as np

import concourse.tile as tile
import concourse.mybir as mybir
from concourse import bacc, bass_utils

B, S, D, F, E, TOPK = 4, 2048, 1024, 4096, 8, 2
T = B * S
P = 128
FT = F // P  # 32 f tiles
DT = D // P  # 8 d tiles
DH = D // 512  # 2 output halves
F32 = mybir.dt.float32
F32R = mybir.dt.float32r
AF = mybir.ActivationFunctionType

_CACHE: dict[tuple, object] = {}


def _block_sizes(n_pad: int) -> list[int]:
    """Decompose n_pad (multiple of 128, >=256) into blocks of 384 and 256."""
    rem = n_pad % 384
    if rem == 0:
        return [384] * (n_pad // 384)
    if rem == 128:
        assert n_pad >= 512
        return [384] * (n_pad // 384 - 1) + [256, 256]
    return [384] * (n_pad // 384) + [256]


def _build(n_pad: int):
    """Build + compile the per-core Bass program for n_pad dispatched tokens."""
    sizes = _block_sizes(n_pad)
    nc = bacc.Bacc("TRN2", target_bir_lowering=False, debug=False)

    # host-prearranged layouts: one fat contiguous chunk per partition
    xT = nc.dram_tensor("xT", (P, DT, n_pad), F32R, kind="ExternalInput")
    w1 = nc.dram_tensor("w1", (P, FT, DT, P), F32R, kind="ExternalInput")
    b1c = nc.dram_tensor("b1c", (P, FT), F32, kind="ExternalInput")
    w2 = nc.dram_tensor("w2", (F, D), F32R, kind="ExternalInput")
    gt = nc.dram_tensor("gt", (P, n_pad // P), F32, kind="ExternalInput")
    y = nc.dram_tensor("y", (n_pad, D), F32, kind="ExternalOutput")

    w2_t = w2.rearrange("(o p) d -> p o d", p=P)  # [128, 32, 1024]

    PREF = 2  # next-block w1 tiles prefetched during the previous block

    with tile.TileContext(nc) as tc:
        with (
            tc.tile_pool(name="w2p", bufs=FT) as w2p,
            tc.tile_pool(name="const", bufs=1) as constp,
            tc.tile_pool(name="xp", bufs=2 * DT) as xp,
            tc.tile_pool(name="w1p", bufs=6) as w1p,
            tc.tile_pool(name="hp", bufs=4) as hp,
            tc.tile_pool(name="op", bufs=4) as op,
            tc.tile_pool(name="ph", bufs=2, space="PSUM") as php,
            tc.tile_pool(name="py", bufs=6, space="PSUM") as pyp,
        ):
            # w2 stays resident; each 512KiB chunk is loaded inside block 0's
            # f-loop, a few iterations ahead of its first use, so it doesn't
            # starve the PE.
            w2_sb = [None] * FT
            W2_AHEAD = 3

            def emit_x(tok, tb, first=0):
                xs = [xp.tile([P, 512], F32R, name="xsb") for _ in range(DT)]
                for d in list(range(first, DT)) + list(range(first)):
                    nc.sync.dma_start(xs[d][:, :tb], xT[:, d, tok : tok + tb])
                return xs

            def emit_w1(f):
                t = w1p.tile([P, DT, P], F32R)
                h = DT // 2
                nc.sync.dma_start(t[:, :h], w1[:, f, :h])
                nc.sync.dma_start(t[:, h:], w1[:, f, h:])
                return t

            def emit_w2(f):
                w2f = w2p.tile([P, D], F32R, name="w2sb")
                nc.sync.dma_start(w2f[:], w2_t[:, f])
                w2_sb[f] = w2f

            # prologue: critical-path DMAs first (x d=0, w1 f=0), then the rest
            xs0 = [xp.tile([P, 512], F32R, name="xsb") for _ in range(DT)]
            nc.sync.dma_start(xs0[0][:, : sizes[0]], xT[:, 0, : sizes[0]])
            w1_pref = [emit_w1(0)]
            for d in range(1, DT):
                nc.sync.dma_start(xs0[d][:, : sizes[0]], xT[:, d, : sizes[0]])
            x_cur = xs0
            w1_pref.append(emit_w1(1))
            b1_sb = constp.tile([P, FT], F32)
            nc.sync.dma_start(b1_sb[:], b1c[:])
            g_sb = constp.tile([P, n_pad // P], F32)
            nc.sync.dma_start(g_sb[:], gt[:])
            for f in range(W2_AHEAD):
                emit_w2(f)

            psum_map: dict[int, list] = {}

            def consume_mm2(carry):
                """Emit the MM2s for step (blk, f) — pipelined one step late so
                the ht LDWEIGHTS never stalls the PE on the RELU result —
                followed by the block epilogue after its final f."""
                cblk, cf, cht, ctb, ctok = carry
                cnt = ctb // P
                if cf == 0:
                    psum_map[cblk] = [
                        pyp.tile([P, 512], F32, name="py") for _ in range(cnt * DH)
                    ]
                ps = psum_map[cblk]
                for t in range(cnt):
                    for dh in range(DH):
                        nc.tensor.matmul(
                            ps[t * DH + dh][:],
                            cht[:, t * P : (t + 1) * P],
                            w2_sb[cf][:, dh * 512 : (dh + 1) * 512],
                            start=(cf == 0),
                            stop=(cf == FT - 1),
                        )
                if cf == FT - 1:
                    for t in range(cnt):
                        col = ctok // P + t
                        for dh in range(DH):
                            pj = ps[t * DH + dh]
                            ot = op.tile([P, 512], F32)
                            if (t * DH + dh) % 2 == 0:
                                nc.scalar.activation(
                                    ot[:], pj[:], AF.Copy,
                                    scale=g_sb[:, col : col + 1],
                                )
                            else:
                                nc.vector.tensor_scalar_mul(
                                    ot[:], pj[:], g_sb[:, col : col + 1]
                                )
                            nc.sync.dma_start(
                                y[
                                    ctok + t * P : ctok + (t + 1) * P,
                                    dh * 512 : (dh + 1) * 512,
                                ],
                                ot[:],
                            )
                    del psum_map[cblk]

            carry = None
            tok = 0
            for blk, tb in enumerate(sizes):
                x_sb = x_cur
                w1_cur, w1_pref = w1_pref, []

                for f in range(FT):
                    w1_sb = w1_cur.pop(0) if w1_cur else emit_w1(f)
                    if blk == 0 and f + W2_AHEAD < FT:
                        emit_w2(f + W2_AHEAD)
                    if blk + 1 < len(sizes):
                        if f == FT - 16:
                            x_cur = emit_x(tok + tb, sizes[blk + 1])
                        elif f >= FT - PREF:
                            w1_pref.append(emit_w1(f - (FT - PREF)))
                    ph = php.tile([P, 512], F32, name="ph")
                    for d in range(DT):
                        nc.tensor.matmul(
                            ph[:, :tb],
                            w1_sb[:, d],
                            x_sb[d][:, :tb],
                            start=(d == 0),
                            stop=(d == DT - 1),
                        )
                    ht = hp.tile([P, 512], F32R, name="ht")
                    nc.scalar.activation(
                        ht[:, :tb], ph[:, :tb], AF.Relu,
                        bias=b1_sb[:, f : f + 1], scale=1.0,
                    )
                    if carry is not None:
                        consume_mm2(carry)
                    carry = (blk, f, ht, tb, tok)
                tok += tb
            consume_mm2(carry)
    nc.compile()
    return nc


def _route(x_flat, Wg, bg):
    """Top-2 routing. Returns (order, counts, offsets, pair gate/idx arrays, n_pad)."""
    logits = x_flat @ Wg + bg  # [T, E]
    i1 = np.argmax(logits, axis=1)
    v1 = logits[np.arange(T), i1]
    masked = logits.copy()
    masked[np.arange(T), i1] = -np.inf
    i2 = np.argmax(masked, axis=1)
    v2 = masked[np.arange(T), i2]
    # softmax over the two selected logits
    e2 = np.exp(v2 - v1)
    g1 = 1.0 / (1.0 + e2)
    g2 = e2 / (1.0 + e2)
    eid = np.stack([i1, i2], 1).reshape(-1)  # [2T]
    gates = np.stack([g1, g2], 1).reshape(-1).astype(np.float32)
    order = np.argsort(eid, kind="stable")
    counts = np.bincount(eid, minlength=E)
    offsets = np.concatenate([[0], np.cumsum(counts)])
    n_pad = max(256, int(-(-counts.max() // P)) * P)
    return order, counts, offsets, gates, n_pad


def kernel(x, Wg, bg, W1, b1, W2, b2, _trace=False):
    x = np.ascontiguousarray(np.asarray(x, dtype=np.float32))
    Wg = np.asarray(Wg, dtype=np.float32)
    bg = np.asarray(bg, dtype=np.float32)
    W1 = np.asarray(W1, dtype=np.float32)
    b1 = np.asarray(b1, dtype=np.float32)
    W2 = np.asarray(W2, dtype=np.float32)
    b2 = np.asarray(b2, dtype=np.float32)

    x_flat = x.reshape(T, D)
    order, counts, offsets, gates, n_pad = _route(x_flat, Wg, bg)

    if n_pad not in _CACHE:
        _CACHE[n_pad] = _build(n_pad)
    nc = _CACHE[n_pad]

    in_maps = []
    for e in range(E):
        ce = int(counts[e])
        sel = order[offsets[e] : offsets[e] + ce]
        toks = sel >> 1
        xd = np.zeros((n_pad, D), dtype=np.float32)
        xd[:ce] = x_flat[toks]
        # [n, d] -> [p, o, n] with d = o*P + p
        xT_e = np.ascontiguousarray(xd.reshape(n_pad, DT, P).transpose(2, 1, 0))
        # [d, f] -> [p, ft, o, m] with d = o*P + p, f = ft*P + m
        w1_e = np.ascontiguousarray(
            W1[e].reshape(DT, P, FT, P).transpose(1, 2, 0, 3)
        )
        g_e = np.zeros(n_pad, dtype=np.float32)
        g_e[:ce] = gates[sel]
        in_maps.append(
            {
                "xT": xT_e,
                "w1": w1_e,
                "b1c": np.ascontiguousarray(b1[e].reshape(FT, P).T),
                "w2": np.ascontiguousarray(W2[e]),
                "gt": np.ascontiguousarray(g_e.reshape(n_pad // P, P).T),
            }
        )

    res = bass_utils.run_bass_kernel_spmd(
        nc, in_maps, core_ids=list(range(E)), trace=_trace
    )

    buf = np.zeros((2 * T, D), dtype=np.float32)
    for e in range(E):
        ce = int(counts[e])
        sel = order[offsets[e] : offsets[e] + ce]
        buf[sel] = res.results[e]["y"][:ce]
    out = buf[0::2] + buf[1::2]
    # b2 is applied host-side: out_t += g1*b2[e1] + g2*b2[e2]
    g_pairs = gates.reshape(T, 2)
    # recover expert ids per pair from the order/offsets partition
    eid_flat = np.empty(2 * T, dtype=np.int64)
    for e in range(E):
        eid_flat[order[offsets[e] : offsets[e + 1]]] = e
    i_pairs = eid_flat.reshape(T, 2)
    out += g_pairs[:, 0:1] * b2[i_pairs[:, 0]] + g_pairs[:, 1:2] * b2[i_pairs[:, 1]]
    if _trace:
        return out.reshape(B, S, D), res
    return out.reshape(B, S, D)
